# revision 1
# baseline (speedup 1.0000x reference)
"""Trainium2 Bass kernel for the CRF negative-log-likelihood loss.

Problem: nn_CRF_73315091742818  (S, B, H, T) = (512, 128, 512, 48)

    emissions = word_features @ W.T + b                  # [S,B,T]
    nll = mean_b( logZ(emissions, transitions) - gold_score )

Strategy (8 NeuronCores, data-parallel over batch, 16 examples/core):

 *  The CRF loss is invariant to adding any per-(s,b) constant to the
    emissions.  We exploit this by centering the emission weights on the
    host (W' = W - mean_t W, b' = b - mean(b) - C with C an empirical
    logsumexp constant).  With g = exp(emis'), the *linear-domain* scaled
    forward recursion
         F_s = (E^T F_{s-1}) * g_s          E = exp(transitions)
    then stays within e^+-15 in fp32 for all 512 steps with NO runtime
    renormalization, and the shift cancels exactly between log Z and the
    gold score:
         nll_b = log(sum_j F_j) - sum_s emis'[tags_sb] - sum_s trans[gold pairs]
 *  Emissions are computed on the tensor engine in bf16 (fp32 PSUM
    accumulation) from a host-pre-transposed [H, S*Bc] activation layout;
    the fp32->bf16 cast happens inside the DMA (SWDGE cast).
 *  The 511-step serial recursion is split into a forward chain (s:0->255)
    and an independent backward chain (s:511->256, beta recursion) that
    run concurrently, meeting in the middle:  Z = sum_j F_255 * beta_255.
    The two chains live in disjoint PE quadrants / partition ranges
    (fwd: partitions 0-47, tile (0,0); bwd: partitions 64-111, tile
    (64,64)) so both transition-weight sets stay resident.
 *  Gold scores are computed on-device from a host-built one-hot tag
    encoding: emis'_gold via masked reduction of ln(g); transition gold
    scores via X = trans @ OH on the PE and a masked reduction against
    the shifted one-hot.
"""

import sys

for _p in ("/opt/trn_rl_repo",):
    if _p not in sys.path:
        sys.path.insert(0, _p)

import numpy as np
import ml_dtypes

S, B, H, T = 512, 128, 512, 48
NCORES = 8
BC = B // NCORES            # 16 examples per core
MEET = 256                  # fwd covers s<MEET, bwd covers s>=MEET
SC = 32                     # s-steps per bulk chunk
CN = SC * BC                # 512 columns per bulk chunk
NCH = MEET // SC            # 8 chunks per half
NB = S * BC                 # 8192 columns per half-pair total
HB = MEET * BC              # 4096 columns per half

_BUILT = None               # cached (nc,) so repeat kernel() calls reuse IR


def _build(mode="full"):
    # mode: "bulk" (emissions+exp only), "rec" (+recursion/combine),
    #       "full" (+gold scores)
    import concourse.bacc as bacc
    import concourse.mybir as mybir
    from concourse.tile import TileContext

    fp32 = mybir.dt.float32
    bf16 = mybir.dt.bfloat16
    AF = mybir.ActivationFunctionType
    ALU = mybir.AluOpType

    nc = bacc.Bacc()

    # ---------------- DRAM I/O ----------------
    wfT = nc.dram_tensor("wft", [H, NB], fp32, kind="ExternalInput")
    wpt = nc.dram_tensor("wpt", [H, T], bf16, kind="ExternalInput")
    bp = nc.dram_tensor("bp", [T, 1], fp32, kind="ExternalInput")
    trans = nc.dram_tensor("trans", [T, T], fp32, kind="ExternalInput")
    transT = nc.dram_tensor("transt", [T, T], fp32, kind="ExternalInput")
    transTb = nc.dram_tensor("transtb", [T, T], bf16, kind="ExternalInput")
    oh = nc.dram_tensor("oh", [T, NB], bf16, kind="ExternalInput")
    out = nc.dram_tensor("out", [1, 64], fp32, kind="ExternalOutput")

    P64 = 64        # base partition of the bwd world
    PB = P64 + T    # 112

    with TileContext(nc) as tc:
        with (
            tc.tile_pool(name="const", bufs=1) as cpool,
            tc.tile_pool(name="big", bufs=1) as gpool,
            tc.tile_pool(name="stage", bufs=32) as spool,
            tc.tile_pool(name="work", bufs=4) as wpool,
            tc.tile_pool(name="scr", bufs=2) as scpool,
            tc.tile_pool(name="ps", bufs=1, space="PSUM") as ppool,
        ):
            # ---------------- constants ----------------
            wpt_sb = cpool.tile([128, 4 * T], bf16, name="wpt_sb")
            for k in range(4):
                nc.sync.dma_start(
                    out=wpt_sb[:, k * T:(k + 1) * T],
                    in_=wpt[k * 128:(k + 1) * 128, :])

            bp0 = cpool.tile([T, 1], fp32, name="bp0")
            nc.sync.dma_start(out=bp0[:], in_=bp[:, :])
            bp64f = cpool.tile([PB, 1], fp32, name="bp64f")
            bp64 = bp64f[P64:PB, :]
            nc.sync.dma_start(out=bp64, in_=bp[:, :])

            traw0 = cpool.tile([T, T], fp32, name="traw0")
            nc.sync.dma_start(out=traw0[:], in_=trans[:, :])
            E0 = cpool.tile([T, T], fp32, name="E0")
            nc.scalar.activation(E0[:], traw0[:], AF.Exp)

            traw64f = cpool.tile([PB, T], fp32, name="traw64f")
            traw64 = traw64f[P64:PB, :]
            nc.sync.dma_start(out=traw64, in_=transT[:, :])
            ET64f = cpool.tile([PB, T], fp32, name="ET64f")
            ET64 = ET64f[P64:PB, :]
            nc.scalar.activation(ET64, traw64, AF.Exp)

            ttb0 = cpool.tile([T, T], bf16, name="ttb0")
            nc.sync.dma_start(out=ttb0[:], in_=transTb[:, :])
            ttb64f = cpool.tile([PB, T], bf16, name="ttb64f")
            ttb64 = ttb64f[P64:PB, :]
            nc.sync.dma_start(out=ttb64, in_=transTb[:, :])

            ones0 = cpool.tile([T, 1], fp32, name="ones0")
            nc.vector.memset(ones0[:], 1.0)
            ones64f = cpool.tile([PB, 1], fp32, name="ones64f")
            ones64 = ones64f[P64:PB, :]
            nc.vector.memset(ones64, 1.0)

            # one-hot tags (bf16), per half
            ohA = gpool.tile([T, HB], bf16, name="ohA")
            nc.sync.dma_start(out=ohA[:], in_=oh[:, 0:HB])
            ohBf = gpool.tile([PB, HB], bf16, name="ohBf")
            ohB = ohBf[P64:PB, :]
            nc.sync.dma_start(out=ohB, in_=oh[:, HB:NB])

            # big persistent activations
            gA = gpool.tile([T, HB], fp32, name="gA")
            gBf = gpool.tile([PB, HB], fp32, name="gBf")
            gB = gBf[P64:PB, :]

            # gold-score accumulators
            egA = cpool.tile([T, BC], fp32, name="egA")
            trA = cpool.tile([T, BC], fp32, name="trA")
            egBf = cpool.tile([PB, BC], fp32, name="egBf")
            egB = egBf[P64:PB, :]
            trBf = cpool.tile([PB, BC], fp32, name="trBf")
            trB = trBf[P64:PB, :]
            for t_ in (egA[:], trA[:], egB, trB):
                nc.vector.memset(t_, 0.0)

            # ---------------- bulk: emissions -> g ----------------
            def bulk_chunk(half, c):
                # half 0: s in [c*SC, (c+1)*SC); half 1: s in [MEET + ...)
                col0 = half * HB + c * CN
                ps = ppool.tile([PB, CN], fp32, name="eps", tag="bulk", bufs=2)
                psv = ps[0:T, :] if half == 0 else ps[P64:PB, :]
                for k in range(4):
                    st = spool.tile([128, CN], bf16, name="st", tag="wfst")
                    nc.gpsimd.dma_start(
                        out=st[:], in_=wfT[k * 128:(k + 1) * 128, col0:col0 + CN])
                    nc.tensor.matmul(
                        psv, wpt_sb[:, k * T:(k + 1) * T], st[:],
                        start=(k == 0), stop=(k == 3), skip_group_check=True)
                gdst = gA[:, c * CN:(c + 1) * CN] if half == 0 \
                    else gB[:, c * CN:(c + 1) * CN]
                bias = bp0[:] if half == 0 else bp64
                nc.scalar.activation(gdst, psv, AF.Exp, bias=bias)

            for c in range(NCH):
                bulk_chunk(0, c)
                bulk_chunk(1, NCH - 1 - c)

            if mode == "bulk":
                finb = cpool.tile([1, 64], fp32, name="finb")
                nc.vector.tensor_copy(finb[:, :], gA[0:1, 0:64])
                nc.sync.dma_start(out=out[:, :], in_=finb[:, :])

            # ---------------- gold-score chunk machinery ----------------
            def gold_chunk(half, c):
                col0 = c * CN
                g_, oh_, eg_, tr_, ttb_ = (
                    (gA[:], ohA[:], egA[:], trA[:], ttb0[:]) if half == 0
                    else (gB, ohB, egB, trB, ttb64))

                def sl(t_, a, b_):
                    return t_[:, a:b_] if half == 0 \
                        else t_  # placeholder, not used

                # ln(g) chunk  (ACT), then masked-reduce for emission gold
                if half == 0:
                    ln = scpool.tile([T, CN], fp32, name="ln", tag="lnA")
                    lnv = ln[:]
                    me = scpool.tile([T, CN], fp32, name="me", tag="meA")
                    mev = me[:]
                    red = scpool.tile([T, BC], fp32, name="red", tag="redA")
                    redv = red[:]
                    mt = scpool.tile([T, CN], fp32, name="mt", tag="mtA")
                    mtv = mt[:]
                else:
                    ln = scpool.tile([PB, CN], fp32, name="ln", tag="lnB")
                    lnv = ln[P64:PB, :]
                    me = scpool.tile([PB, CN], fp32, name="me", tag="meB")
                    mev = me[P64:PB, :]
                    red = scpool.tile([PB, BC], fp32, name="red", tag="redB")
                    redv = red[P64:PB, :]
                    mt = scpool.tile([PB, CN], fp32, name="mt", tag="mtB")
                    mtv = mt[P64:PB, :]

                gch = g_[:, col0:col0 + CN]
                ohch = oh_[:, col0:col0 + CN]
                # NB: gpsimd works standalone here but crashes the exec unit
                # when its tensor ops overlap the X-matmul chain; keep on DVE.
                mult_eng = nc.gpsimd if mode == "g_gp" else nc.vector
                nc.scalar.activation(lnv, gch, AF.Ln)
                mult_eng.tensor_tensor(mev, lnv, ohch, ALU.mult)
                nc.vector.tensor_reduce(
                    redv, mev.rearrange("p (s b) -> p b s", b=BC),
                    axis=mybir.AxisListType.X, op=ALU.add)
                nc.vector.tensor_tensor(eg_, eg_, redv, ALU.add)

                if mode == "g_noX":
                    return
                # transition gold:  X = trans @ OH ;  reduce(X_s * OH_{s-1})
                xps = ppool.tile([PB, CN], fp32, name="xps", tag="bulk", bufs=2)
                xpsv = xps[0:T, :] if half == 0 else xps[P64:PB, :]
                nc.tensor.matmul(xpsv, ttb_, ohch, skip_group_check=True)
                if c == 0:
                    xin = xpsv[:, BC:CN]
                    ohin = oh_[:, 0:CN - BC]
                    nred = SC - 1
                else:
                    xin = xpsv[:, :]
                    ohin = oh_[:, col0 - BC:col0 + CN - BC]
                    nred = SC
                nc.vector.tensor_tensor(mtv[:, 0:nred * BC], xin, ohin, ALU.mult)
                nc.vector.tensor_reduce(
                    redv, mtv[:, 0:nred * BC].rearrange("p (s b) -> p b s", b=BC),
                    axis=mybir.AxisListType.X, op=ALU.add)
                nc.vector.tensor_tensor(tr_, tr_, redv, ALU.add)

            # ---------------- recursion ----------------
            # fwd state: F_s [T, BC] @p0 ; bwd state beta_s [T, BC] @p64
            f_prev = gA[:, 0:BC]
            b_prev = gB[:, (MEET - 1) * BC:MEET * BC]   # u_0 = g_511
            fps_prev = None
            bps_prev = None

            for r in range(MEET if mode != "bulk" else 0):
                # backward step t=r: beta_{511-r-1} = E @ u_r
                bps = ppool.tile([PB, BC], fp32, name="bps", tag="brec", bufs=3)
                nc.tensor.matmul(bps[P64:PB, :], ET64, b_prev,
                                 skip_group_check=True)
                bps_prev = bps
                if r < MEET - 1:
                    # u_{r+1} = beta_{511-r-1} * g_{511-r-1}
                    un = wpool.tile([PB, BC], fp32, name="un", tag="u")
                    scol = (MEET - 2 - r) * BC
                    nc.vector.tensor_tensor(
                        un[P64:PB, :], bps[P64:PB, :],
                        gB[:, scol:scol + BC], ALU.mult)
                    b_prev = un[P64:PB, :]

                # forward step s=r (r>=1): F_r = (E^T F_{r-1}) * g_r
                if r >= 1:
                    fps = ppool.tile([T, BC], fp32, name="fps", tag="frec",
                                     bufs=3)
                    nc.tensor.matmul(fps[:], E0[:], f_prev,
                                     skip_group_check=True)
                    fn = wpool.tile([T, BC], fp32, name="fn", tag="f")
                    nc.vector.tensor_tensor(
                        fn[:], fps[:], gA[:, r * BC:(r + 1) * BC], ALU.mult)
                    f_prev = fn[:]

                # sprinkle gold-score chunks into recursion gaps
                if mode in ("full", "g_noX", "g_dve"):
                    if r % SC == 16 and r // SC < NCH:
                        gold_chunk(0, r // SC)
                    if r % SC == 24 and r // SC < NCH:
                        gold_chunk(1, NCH - 1 - r // SC)

            # cross transition term s=256:  Xc = trans @ OH[:,256] vs OH[:,255]
            if mode in ("full", "g_dve"):
                xc = ppool.tile([T, BC], fp32, name="xc", tag="frec", bufs=3)
                nc.tensor.matmul(xc[:], ttb64, ohB[:, 0:BC],
                                 tile_position=(P64, 0), skip_group_check=True)
                mc = wpool.tile([T, BC], fp32, name="mc", tag="f")
                nc.vector.tensor_tensor(
                    mc[:], xc[:], ohA[:, (MEET - 1) * BC:MEET * BC], ALU.mult)
                nc.vector.tensor_tensor(trA[:], trA[:], mc[:], ALU.add)

            # ---------------- combine ----------------
            if mode != "bulk":
                # move beta_255 (psum @p64) down to partitions 0-47
                ub = wpool.tile([PB, BC], fp32, name="ub", tag="u")
                nc.vector.tensor_copy(ub[P64:PB, :], bps_prev[P64:PB, :])
                betam = cpool.tile([T, BC], fp32, name="betam")
                nc.sync.dma_start(out=betam[:], in_=ub[P64:PB, :])

                pm = cpool.tile([T, BC], fp32, name="pm")
                nc.vector.tensor_tensor(pm[:], f_prev, betam[:], ALU.mult)

                sig = ppool.tile([1, BC], fp32, name="sig", tag="frec", bufs=3)
                nc.tensor.matmul(sig[:], ones0[:], pm[:], skip_group_check=True)

                fin = cpool.tile([1, 64], fp32, name="fin")
                if mode in ("full", "g_noX", "g_dve"):
                    # gold = sum_t (egA+trA) + sum_t (egB+trB); the two
                    # partition worlds get separate single-matmul groups
                    # (a cross-quadrant accumulation group breaks on HW).
                    nc.vector.tensor_tensor(egA[:], egA[:], trA[:], ALU.add)
                    nc.vector.tensor_tensor(egB, egB, trB, ALU.add)
                    pgA = ppool.tile([1, BC], fp32, name="pgA", tag="frec",
                                     bufs=3)
                    nc.tensor.matmul(pgA[:], ones0[:], egA[:],
                                     skip_group_check=True)
                    pgB = ppool.tile([1, BC], fp32, name="pgB", tag="brec",
                                     bufs=3)
                    nc.tensor.matmul(pgB[:], ones64, egB,
                                     tile_position=(P64, 0),
                                     skip_group_check=True)
                    nc.vector.tensor_copy(fin[:, BC:2 * BC], pgA[:])
                    nc.vector.tensor_tensor(fin[:, BC:2 * BC],
                                            fin[:, BC:2 * BC], pgB[:],
                                            ALU.add)
                else:
                    nc.vector.memset(fin[:, BC:2 * BC], 0.0)
                nc.scalar.activation(fin[:, 0:BC], sig[:], AF.Ln)
                nc.vector.tensor_tensor(
                    fin[:, 2 * BC:3 * BC], fin[:, 0:BC], fin[:, BC:2 * BC],
                    ALU.subtract)
                nc.vector.memset(fin[:, 3 * BC:64], 0.0)
                nc.sync.dma_start(out=out[:, :], in_=fin[:, :])

    # Bacc lowering (register allocation, >1-sync-wait splitting, ...) runs
    # in finalize(); run_bass_via_pjrt serializes nc as-is, so do it here.
    nc.finalize()
    return nc


def _host_prep(word_features, W, b, transitions, tags):
    wf = np.ascontiguousarray(word_features, dtype=np.float32)
    W = np.asarray(W, np.float32)
    b = np.asarray(b, np.float32)
    trans = np.ascontiguousarray(transitions, dtype=np.float32)
    tags = np.asarray(tags)

    wbar = W.mean(axis=0)
    bbar = b.mean()
    Wp = W - wbar[None, :]
    # empirical per-step logsumexp constant (keeps the scaled recursion ~O(1))
    rng = np.random.default_rng(0)
    ss = rng.integers(0, S, 64)
    bs = rng.integers(0, B, 64)
    sample = wf[ss, bs, :] @ Wp.T + (b - bbar)[None, :]
    m = sample.max(axis=1, keepdims=True)
    C = float(np.mean(m + np.log(np.exp(sample - m).sum(axis=1))))
    bp = (b - bbar - C).astype(np.float32).reshape(T, 1)

    wptb = np.ascontiguousarray(Wp.T).astype(ml_dtypes.bfloat16)  # [H, T]
    transT = np.ascontiguousarray(trans.T)
    transTb = transT.astype(ml_dtypes.bfloat16)

    tgs = tags.astype(np.int64)
    in_maps = []
    for c in range(NCORES):
        bsl = slice(c * BC, (c + 1) * BC)
        wfT_c = np.ascontiguousarray(
            wf[:, bsl, :].transpose(2, 0, 1).reshape(H, NB))
        tg_c = tgs[:, bsl]                                   # [S, BC]
        oh_c = (tg_c[None, :, :] == np.arange(T)[:, None, None])
        oh_c = np.ascontiguousarray(
            oh_c.reshape(T, NB)).astype(ml_dtypes.bfloat16)
        in_maps.append({
            "wft": wfT_c, "wpt": wptb, "bp": bp, "trans": trans,
            "transt": transT, "transtb": transTb, "oh": oh_c,
        })
    return in_maps


def kernel(word_features, W, b, transitions, tags):
    global _BUILT
    if _BUILT is None:
        _BUILT = _build()
    nc = _BUILT

    from concourse.bass_utils import run_bass_kernel_spmd

    in_maps = _host_prep(word_features, W, b, transitions, tags)
    res = run_bass_kernel_spmd(nc, in_maps, core_ids=list(range(NCORES)))
    parts = [r["out"].reshape(64)[2 * BC:3 * BC] for r in res.results]
    nll = np.concatenate(parts).astype(np.float32).mean()
    return np.float32(nll)


if __name__ == "__main__":
    # smoke test: build only
    nc = _build()
    print("build OK")

